# revision 18
# baseline (speedup 1.0000x reference)
"""Trainium2 Bass kernel for BatchedLonCtrl (retrieval_knn).

Contract: kernel(**inputs) takes the FULL unsharded inputs (as produced by
setup_inputs()) and returns the FULL [B] float32 output. Internally the batch
dim is sharded across 8 NeuronCores (pure data parallel), the Bass program is
compiled once and run via run_bass_kernel_spmd.

Device algorithm per core (512 rows = 4 chunks x 128 partitions):
  1. stream ref_x, ref_y(masked), ref_t row-chunks into SBUF
  2. dist2 = (rx-x)^2 + (ry-y)^2 via ACT Square + DVE add / min-reduce
     (valid_mask is pre-folded into ref_y on host: invalid -> 1e9 -> dist2 ~1e18)
  3. argmin index via DVE max_index (value matcher) on the min value
  4. gather (t,v,a,s)[idx] via indirect DMA from a host-packed [T,4] interleave
  5. searchsorted(ref_t, t_cl) as a count of (ref_t < t_cl); either a DVE
     is_lt+accum pass or an ACT Sign+accum pass with an exact fixup
  6. gather (t,v,a,s)[ii], (t,v,a,s)[ii+1] in one 8-wide indirect DMA
  7. linear interp + station/speed PID + clamps, batched [128,4] per core
"""

import numpy as np

try:
    import concourse.bass as bass
except ImportError:  # environment provides the repo at /opt/trn_rl_repo
    import sys

    sys.path.insert(0, "/opt/trn_rl_repo")
    import concourse.bass as bass

import concourse.bacc as bacc
import concourse.tile as tile
from concourse import mybir
from concourse.bass import IndirectOffsetOnAxis
from concourse.bass_utils import run_bass_kernel_spmd

F32 = mybir.dt.float32
I32 = mybir.dt.int32
U32 = mybir.dt.uint32
AF = mybir.ActivationFunctionType
OP = mybir.AluOpType

B, T = 4096, 2048
NCORES = 8
RPC = B // NCORES  # rows per core = 512
P = 128
CH = RPC // P  # chunks per core = 4

DT = 0.02
PREVIEW_WINDOW = 0.8
STATION_ERR_LIM = 5.0
SPEED_INPUT_LIM = 3.0
INTEGRATOR_SAT = 5.0
ACC_MIN, ACC_MAX = -4.0, 2.0
MASK_BIG = 1.0e9  # invalid ref_y replacement; dist2 becomes ~1e18 >> any valid

# feature flags (validated per-op on HW)
USE_SIGN_COUNT = True  # searchsorted count via ACT Sign+accum instead of DVE
USE_MAXBC = True  # broadcast minv AP directly into max_index

_CACHE = {}


def _build_program():
    if "nc" in _CACHE:
        return _CACHE["nc"]

    nc = bacc.Bacc(
        "TRN2", target_bir_lowering=False, debug=False, enable_asserts=False
    )

    rx_d = nc.dram_tensor("rx", [RPC, T], F32, kind="ExternalInput").ap()
    ym_d = nc.dram_tensor("ym", [RPC, T], F32, kind="ExternalInput").ap()
    rt_d = nc.dram_tensor("rt", [RPC, T], F32, kind="ExternalInput").ap()
    tvas_d = nc.dram_tensor("tvas", [RPC * T, 4], F32, kind="ExternalInput").ap()
    vec_d = nc.dram_tensor("vec", [P, 32], F32, kind="ExternalInput").ap()
    out_d = nc.dram_tensor("out", [P, CH], F32, kind="ExternalOutput").ap()

    # vec columns:
    #  0: 4   -x per chunk          4: 8   -y per chunk      8:12  v per chunk
    # 12:16   t_max per chunk      16:20   integral_station  20:24 integral_speed
    # 24 kp5=5*station_kp  25 station_ki  26 lokp3=3*low_kp  27 low_ki
    # 28 dkp3=3*(high_kp-low_kp)  29 dki=high_ki-low_ki  30 -2*switch_speed

    with tile.TileContext(nc) as tc:
        from contextlib import ExitStack

        with ExitStack() as ctx:
            singles = ctx.enter_context(tc.tile_pool(name="singles", bufs=1))
            stream = ctx.enter_context(tc.tile_pool(name="stream", bufs=2))
            work = ctx.enter_context(tc.tile_pool(name="work", bufs=2))
            small = ctx.enter_context(tc.tile_pool(name="small", bufs=2))
            accp = ctx.enter_context(tc.tile_pool(name="accp", bufs=1))

            vec = singles.tile([P, 32], F32)
            nc.sync.dma_start(out=vec[:], in_=vec_d)

            # rbcu[:, c] = p*T + c*128*T  (tvas row base, uint32)
            rbcu = singles.tile([P, CH], U32)
            for c in range(CH):
                nc.gpsimd.iota(
                    rbcu[:, c : c + 1],
                    pattern=[[1, 1]],
                    base=c * P * T,
                    channel_multiplier=T,
                )

            # per-core accumulators
            idx_all = accp.tile([P, 8 * CH], U32)  # FIND_INDEX8 outputs
            off1_all = accp.tile([P, CH], U32)
            G1 = accp.tile([P, 4 * CH], F32)  # (t,v,a,s) at idx, per chunk
            S_all = accp.tile([P, CH], F32)
            tcl_all = accp.tile([P, CH], F32)
            off2_all = accp.tile([P, CH], U32)
            G2 = accp.tile([P, 8 * CH], F32)  # (t,v,a,s) at ii, ii+1

            # ---- phase A (per chunk): stream + dist2 + argmin ----
            rt_tiles = []
            for c in range(CH):
                rows = slice(c * P, (c + 1) * P)
                rx_t = stream.tile([P, T], F32, tag="rx")
                nc.sync.dma_start(out=rx_t[:], in_=rx_d[rows])
                ym_t = stream.tile([P, T], F32, tag="ym")
                nc.sync.dma_start(out=ym_t[:], in_=ym_d[rows])
                rt_t = stream.tile([P, T], F32, tag="rt", bufs=CH)
                nc.sync.dma_start(out=rt_t[:], in_=rt_d[rows])
                rt_tiles.append(rt_t)

                dx2 = work.tile([P, T], F32, tag="dx2")
                nc.scalar.activation(
                    dx2[:], rx_t[:], AF.Square, bias=vec[:, c : c + 1], scale=1.0
                )
                dy2 = work.tile([P, T], F32, tag="dy2")
                nc.scalar.activation(
                    dy2[:], ym_t[:], AF.Square, bias=vec[:, 4 + c : 5 + c], scale=1.0
                )

                dist2 = work.tile([P, T], F32, tag="dist2")
                nc.vector.tensor_tensor(
                    out=dist2[:], in0=dx2[:], in1=dy2[:], op=OP.add
                )
                minv = small.tile([P, 1], F32, tag="minv")
                nc.vector.tensor_reduce(
                    out=minv[:], in_=dist2[:], axis=mybir.AxisListType.X, op=OP.min
                )
                nc.vector.max_index(
                    idx_all[:, 8 * c : 8 * c + 8],
                    minv[:, 0:1].to_broadcast([P, 8]),
                    dist2[:],
                )

            # ---- phase B: batched gather1 + t_cl ----
            idx_v = idx_all[:].rearrange("p (c k) -> p c k", k=8)[:, :, 0]
            nc.vector.tensor_tensor(
                out=off1_all[:], in0=idx_v, in1=rbcu[:], op=OP.add
            )
            G1r = G1[:].rearrange("p (c k) -> p c k", k=4)
            for c in range(CH):
                nc.gpsimd.indirect_dma_start(
                    out=G1[:, 4 * c : 4 * c + 4],
                    out_offset=None,
                    in_=tvas_d,
                    in_offset=IndirectOffsetOnAxis(ap=off1_all[:, c : c + 1], axis=0),
                )
            # t_cl = min(t_m + 0.8, t_max)   [t_m >= 0 so the max(.,0) is dead]
            tq = small.tile([P, CH], F32, tag="tq")
            nc.vector.tensor_scalar(
                out=tq[:], in0=G1r[:, :, 0], scalar1=PREVIEW_WINDOW,
                scalar2=None, op0=OP.add,
            )
            nc.vector.tensor_tensor(
                out=tcl_all[:], in0=tq[:], in1=vec[:, 12:16], op=OP.min
            )

            # ---- phase C (per chunk): sign-count pass ----
            for c in range(CH):
                cntscr = work.tile([P, T], F32, tag="dx2")
                nc.scalar.activation(
                    cntscr[:], rt_tiles[c][:], AF.Sign,
                    bias=tcl_all[:, c : c + 1], scale=-1.0,
                    accum_out=S_all[:, c : c + 1],
                )

            # ---- phase D: batched count fixup + ii + offsets ----
            # cnt = f*S + (1-f)*(S+T)/2 with f = (t_cl == t_max); using
            # s = sign(t_cl - t_max) in {-1, 0}: cnt = s*(S - a) + S, a = S/2+T/2
            diff = small.tile([P, CH], F32, tag="diff")
            nc.vector.tensor_tensor(
                out=diff[:], in0=tcl_all[:], in1=vec[:, 12:16], op=OP.subtract
            )
            sflag = small.tile([P, CH], F32, tag="sflag")
            nc.scalar.activation(sflag[:], diff[:], AF.Sign, scale=1.0)
            a_t = small.tile([P, CH], F32, tag="a_t")
            nc.vector.tensor_scalar(
                out=a_t[:], in0=S_all[:], scalar1=0.5, scalar2=float(T // 2),
                op0=OP.mult, op1=OP.add,
            )
            dlt = small.tile([P, CH], F32, tag="dlt")
            nc.vector.tensor_tensor(
                out=dlt[:], in0=S_all[:], in1=a_t[:], op=OP.subtract
            )
            e_t = small.tile([P, CH], F32, tag="e_t")
            nc.vector.tensor_tensor(
                out=e_t[:], in0=sflag[:], in1=dlt[:], op=OP.mult
            )
            cnt = small.tile([P, CH], F32, tag="cnt")
            nc.vector.tensor_tensor(
                out=cnt[:], in0=e_t[:], in1=S_all[:], op=OP.add
            )
            # ii = max(cnt-1, 0)   [cnt <= T-1 always, so no upper clamp]
            ii1 = small.tile([P, CH], F32, tag="ii1")
            nc.vector.tensor_scalar(
                out=ii1[:], in0=cnt[:], scalar1=-1.0, scalar2=0.0,
                op0=OP.add, op1=OP.max,
            )
            iiu = small.tile([P, CH], U32, tag="iiu")
            nc.vector.tensor_copy(iiu[:], ii1[:])
            nc.vector.tensor_tensor(
                out=off2_all[:], in0=iiu[:], in1=rbcu[:], op=OP.add
            )

            # ---- phase E: gather2 (one single-offset indirect DMA per chunk) ----
            G2r = G2[:].rearrange("p (c k) -> p c k", k=8)
            for c in range(CH):
                nc.gpsimd.indirect_dma_start(
                    out=G2[:, 8 * c : 8 * c + 8],
                    out_offset=None,
                    in_=tvas_d,
                    in_offset=IndirectOffsetOnAxis(ap=off2_all[:, c : c + 1], axis=0),
                )

            # ---- phase F: batched frac + interpolation + PID ----
            # frac = clip((t_cl - t0) / (t1 - t0), 0, 1)   [t1-t0 ~ 0.1 > 0]
            t0v = G2r[:, :, 0]
            den = small.tile([P, CH], F32, tag="den")
            nc.vector.tensor_tensor(
                out=den[:], in0=G2r[:, :, 4], in1=t0v, op=OP.subtract
            )
            rec = small.tile([P, CH], F32, tag="rec")
            nc.vector.reciprocal(rec[:], den[:])
            num = small.tile([P, CH], F32, tag="num")
            nc.vector.tensor_tensor(
                out=num[:], in0=tcl_all[:], in1=t0v, op=OP.subtract
            )
            fr = small.tile([P, CH], F32, tag="fr")
            nc.vector.tensor_tensor(out=fr[:], in0=num[:], in1=rec[:], op=OP.mult)
            frac_all = small.tile([P, CH], F32, tag="frac_all")
            nc.vector.tensor_scalar(
                out=frac_all[:], in0=fr[:], scalar1=0.0, scalar2=1.0,
                op0=OP.max, op1=OP.min,
            )

            Dall = accp.tile([P, 4 * CH], F32)
            Dr = Dall[:].rearrange("p (c k) -> p c k", k=4)
            nc.vector.tensor_tensor(
                out=Dr, in0=G2r[:, :, 4:8], in1=G2r[:, :, 0:4], op=OP.subtract
            )
            Pall = accp.tile([P, 4 * CH], F32)
            for c in range(CH):
                nc.vector.tensor_scalar(
                    out=Pall[:, 4 * c : 4 * c + 4],
                    in0=Dall[:, 4 * c : 4 * c + 4],
                    scalar1=frac_all[:, c : c + 1],
                    scalar2=None,
                    op0=OP.mult,
                )
            Iall = accp.tile([P, 4 * CH], F32)
            Ir = Iall[:].rearrange("p (c k) -> p c k", k=4)
            Pr = Pall[:].rearrange("p (c k) -> p c k", k=4)
            nc.vector.tensor_tensor(
                out=Ir, in0=Pr, in1=G2r[:, :, 0:4], op=OP.add
            )

            s_m = G1r[:, :, 3]  # [P, CH] strided view
            v_p = Ir[:, :, 1]
            a_p = Ir[:, :, 2]
            s_p = Ir[:, :, 3]

            def pt(tag):
                return small.tile([P, CH], F32, tag=tag, name=tag)

            # station PI: station_err = 5*tanh((s_p-s_m)/5), folded as th*5
            serr0 = pt("serr0")
            nc.vector.tensor_tensor(out=serr0[:], in0=s_p, in1=s_m, op=OP.subtract)
            th = pt("th")
            nc.scalar.activation(
                th[:], serr0[:], AF.Tanh, scale=float(1.0 / STATION_ERR_LIM)
            )
            t1a = pt("t1a")  # station_err*DT = th*0.1
            nc.scalar.activation(t1a[:], th[:], AF.Identity, scale=0.1)
            ints0 = pt("ints0")
            nc.vector.tensor_tensor(
                out=ints0[:], in0=t1a[:], in1=vec[:, 16:20], op=OP.add
            )
            ints = pt("ints")
            nc.vector.tensor_scalar(
                out=ints[:], in0=ints0[:], scalar1=-INTEGRATOR_SAT,
                scalar2=INTEGRATOR_SAT, op0=OP.max, op1=OP.min,
            )
            so1 = pt("so1")  # station_kp*station_err = th*(5*station_kp)
            nc.scalar.activation(so1[:], th[:], AF.Identity, scale=vec[:, 24:25])
            so2 = pt("so2")
            nc.scalar.activation(so2[:], ints[:], AF.Identity, scale=vec[:, 25:26])
            soff = pt("soff")
            nc.vector.tensor_tensor(out=soff[:], in0=so1[:], in1=so2[:], op=OP.add)

            # speed PI: speed_err = 3*tanh(ve1/3) folded as th2*3
            ve0 = pt("ve0")
            nc.vector.tensor_tensor(out=ve0[:], in0=v_p, in1=soff[:], op=OP.add)
            ve1 = pt("ve1")
            nc.vector.tensor_tensor(
                out=ve1[:], in0=ve0[:], in1=vec[:, 8:12], op=OP.subtract
            )
            th2 = pt("th2")
            nc.scalar.activation(
                th2[:], ve1[:], AF.Tanh, scale=float(1.0 / SPEED_INPUT_LIM)
            )
            t2a = pt("t2a")  # speed_err*DT = th2*0.06
            nc.scalar.activation(t2a[:], th2[:], AF.Identity, scale=0.06)
            insp0 = pt("insp0")
            nc.vector.tensor_tensor(
                out=insp0[:], in0=t2a[:], in1=vec[:, 20:24], op=OP.add
            )
            insp = pt("insp")
            nc.vector.tensor_scalar(
                out=insp[:], in0=insp0[:], scalar1=-INTEGRATOR_SAT,
                scalar2=INTEGRATOR_SAT, op0=OP.max, op1=OP.min,
            )
            w = pt("w")
            nc.scalar.activation(
                w[:], vec[:, 8:12], AF.Sigmoid, bias=vec[:, 30:31], scale=2.0
            )
            kp3 = pt("kp3")  # 3*kp = w*dkp3 + lokp3
            nc.scalar.activation(
                kp3[:], w[:], AF.Identity, scale=vec[:, 28:29], bias=vec[:, 26:27]
            )
            ki = pt("ki")
            nc.scalar.activation(
                ki[:], w[:], AF.Identity, scale=vec[:, 29:30], bias=vec[:, 27:28]
            )
            p1 = pt("p1")  # kp*speed_err = kp3*th2
            nc.vector.tensor_tensor(out=p1[:], in0=kp3[:], in1=th2[:], op=OP.mult)
            p2 = pt("p2")
            nc.vector.tensor_tensor(out=p2[:], in0=ki[:], in1=insp[:], op=OP.mult)
            p3 = pt("p3")
            nc.vector.tensor_tensor(out=p3[:], in0=p1[:], in1=p2[:], op=OP.add)
            p4 = pt("p4")
            nc.vector.tensor_tensor(out=p4[:], in0=p3[:], in1=a_p, op=OP.add)
            accf = pt("accf")
            nc.vector.tensor_scalar(
                out=accf[:], in0=p4[:], scalar1=ACC_MIN, scalar2=ACC_MAX,
                op0=OP.max, op1=OP.min,
            )
            nc.sync.dma_start(out=out_d, in_=accf[:])

    nc.compile()
    _CACHE["nc"] = nc
    return nc


def _prepare_in_maps(inputs):
    def f(name):
        return np.ascontiguousarray(np.asarray(inputs[name], dtype=np.float32))

    rx = f("ref_x")
    ry = f("ref_y")
    rt = f("ref_t")
    valid = f("valid_mask")
    ym = np.where(valid > 0.5, ry, np.float32(MASK_BIG)).astype(np.float32)
    tvas = np.stack(
        [rt, f("ref_v"), f("ref_a"), f("ref_s")], axis=2
    )  # [B, T, 4] contiguous

    xs = f("x")
    ys = f("y")
    vs = f("v")
    tmax = f("t_max")
    ist = f("integral_station")
    isp = f("integral_speed")

    sk = np.float32(np.asarray(inputs["station_kp"]))
    si = np.float32(np.asarray(inputs["station_ki"]))
    lkp = np.float32(np.asarray(inputs["low_speed_kp"]))
    lki = np.float32(np.asarray(inputs["low_speed_ki"]))
    hkp = np.float32(np.asarray(inputs["high_speed_kp"]))
    hki = np.float32(np.asarray(inputs["high_speed_ki"]))
    sw = np.float32(np.asarray(inputs["switch_speed"]))

    in_maps = []
    for core in range(NCORES):
        base = core * RPC
        sl = slice(base, base + RPC)
        vec = np.zeros((P, 32), np.float32)
        for c in range(CH):
            rows = slice(base + c * P, base + (c + 1) * P)
            vec[:, 0 + c] = -xs[rows]
            vec[:, 4 + c] = -ys[rows]
            vec[:, 8 + c] = vs[rows]
            vec[:, 12 + c] = tmax[rows]
            vec[:, 16 + c] = ist[rows]
            vec[:, 20 + c] = isp[rows]
        vec[:, 24] = np.float32(5.0) * sk
        vec[:, 25] = si
        vec[:, 26] = np.float32(3.0) * lkp
        vec[:, 27] = lki
        vec[:, 28] = np.float32(3.0) * (hkp - lkp)
        vec[:, 29] = hki - lki
        vec[:, 30] = np.float32(-2.0) * sw
        in_maps.append(
            {
                "rx": np.ascontiguousarray(rx[sl]),
                "ym": np.ascontiguousarray(ym[sl]),
                "rt": np.ascontiguousarray(rt[sl]),
                "tvas": tvas[sl].reshape(RPC * T, 4),
                "vec": vec,
            }
        )
    return in_maps


def _assemble(results):
    out = np.empty(B, np.float32)
    for core in range(NCORES):
        oc = np.asarray(results[core]["out"], np.float32)  # [P, CH]
        out[core * RPC : (core + 1) * RPC] = oc.T.reshape(RPC)
    return out


def kernel(**inputs):
    nc = _build_program()
    in_maps = _prepare_in_maps(inputs)
    res = run_bass_kernel_spmd(nc, in_maps, core_ids=list(range(NCORES)))
    return _assemble(res.results)


def kernel_traced(inputs, **kwargs):
    """For test.py: same as kernel() but returns (output, BassKernelResults)."""
    nc = _build_program()
    in_maps = _prepare_in_maps(inputs)
    res = run_bass_kernel_spmd(
        nc, in_maps, core_ids=list(range(NCORES)), trace=True, **kwargs
    )
    return _assemble(res.results), res


# revision 25
# speedup vs baseline: 1.0932x; 1.0932x over previous
"""Trainium2 Bass kernel for BatchedLonCtrl (retrieval_knn).

Contract: kernel(**inputs) takes the FULL unsharded inputs (as produced by
setup_inputs()) and returns the FULL [B] float32 output. Internally the batch
dim is sharded across 8 NeuronCores (pure data parallel), the Bass program is
compiled once and run via run_bass_kernel_spmd.

Device algorithm per core (512 rows = 4 chunks x 128 partitions):
  1. stream ref_x, ref_y(masked), ref_t row-chunks into SBUF
  2. dist2 = (rx-x)^2 + (ry-y)^2 via ACT Square + DVE add / min-reduce
     (valid_mask is pre-folded into ref_y on host: invalid -> 1e9 -> dist2 ~1e18)
  3. argmin index via DVE max_index (value matcher) on the min value
  4. gather (t,v,a,s)[idx] via indirect DMA from a host-packed [T,4] interleave
  5. searchsorted(ref_t, t_cl) as a count of (ref_t < t_cl); either a DVE
     is_lt+accum pass or an ACT Sign+accum pass with an exact fixup
  6. gather (t,v,a,s)[ii], (t,v,a,s)[ii+1] in one 8-wide indirect DMA
  7. linear interp + station/speed PID + clamps, batched [128,4] per core
"""

import numpy as np

try:
    import concourse.bass as bass
except ImportError:  # environment provides the repo at /opt/trn_rl_repo
    import sys

    sys.path.insert(0, "/opt/trn_rl_repo")
    import concourse.bass as bass

import concourse.bacc as bacc
import concourse.tile as tile
from concourse import mybir
from concourse.bass import IndirectOffsetOnAxis
from concourse.bass_utils import run_bass_kernel_spmd

F32 = mybir.dt.float32
I32 = mybir.dt.int32
U32 = mybir.dt.uint32
AF = mybir.ActivationFunctionType
OP = mybir.AluOpType

B, T = 4096, 2048
NCORES = 8
RPC = B // NCORES  # rows per core = 512
P = 128
CH = RPC // P  # chunks per core = 4

DT = 0.02
PREVIEW_WINDOW = 0.8
STATION_ERR_LIM = 5.0
SPEED_INPUT_LIM = 3.0
INTEGRATOR_SAT = 5.0
ACC_MIN, ACC_MAX = -4.0, 2.0
MASK_BIG = 1.0e9  # invalid ref_y replacement; dist2 becomes ~1e18 >> any valid

# feature flags (validated per-op on HW)
USE_SIGN_COUNT = True  # searchsorted count via ACT Sign+accum instead of DVE
USE_MAXBC = True  # broadcast minv AP directly into max_index

_CACHE = {}


def _build_program():
    if "nc" in _CACHE:
        return _CACHE["nc"]

    nc = bacc.Bacc(
        "TRN2", target_bir_lowering=False, debug=False, enable_asserts=False
    )

    xym_d = nc.dram_tensor("xym", [RPC, 2, T], F32, kind="ExternalInput").ap()
    rt_d = nc.dram_tensor("rt", [RPC, T], F32, kind="ExternalInput").ap()
    tvas_d = nc.dram_tensor("tvas", [RPC * T, 4], F32, kind="ExternalInput").ap()
    vec_d = nc.dram_tensor("vec", [P, 32], F32, kind="ExternalInput").ap()
    out_d = nc.dram_tensor("out", [P, CH], F32, kind="ExternalOutput").ap()

    # vec columns:
    #  0: 4   -x per chunk          4: 8   -y per chunk      8:12  v per chunk
    # 12:16   t_max per chunk      16:20   integral_station  20:24 integral_speed
    # 24 kp5=5*station_kp  25 station_ki  26 lokp3=3*low_kp  27 low_ki
    # 28 dkp3=3*(high_kp-low_kp)  29 dki=high_ki-low_ki  30 -2*switch_speed

    with tile.TileContext(nc) as tc:
        from contextlib import ExitStack

        with ExitStack() as ctx:
            singles = ctx.enter_context(tc.tile_pool(name="singles", bufs=1))
            stream = ctx.enter_context(tc.tile_pool(name="stream", bufs=2))
            work = ctx.enter_context(tc.tile_pool(name="work", bufs=2))
            small = ctx.enter_context(tc.tile_pool(name="small", bufs=2))
            accp = ctx.enter_context(tc.tile_pool(name="accp", bufs=1))

            vec = singles.tile([P, 32], F32)
            nc.sync.dma_start(out=vec[:], in_=vec_d)

            # rbcu[:, c] = p*T + c*128*T  (tvas row base for gather1, int32)
            # rbm1[:, c] = p*T + c*128*T - 1  (base with the ii=cnt-1 folded in)
            rbcu = singles.tile([P, CH], U32)
            rbm1 = singles.tile([P, CH], I32)
            for c in range(CH):
                nc.gpsimd.iota(
                    rbcu[:, c : c + 1], pattern=[[1, 1]],
                    base=c * P * T, channel_multiplier=T,
                )
                nc.gpsimd.iota(
                    rbm1[:, c : c + 1], pattern=[[1, 1]],
                    base=c * P * T - 1, channel_multiplier=T,
                )

            # per-core accumulators
            idx_all = accp.tile([P, 8 * CH], U32)  # FIND_INDEX8 outputs
            G1 = accp.tile([P, 4 * CH], F32)  # (t,v,a,s) at idx, per chunk
            S_all = accp.tile([P, CH], F32)
            tcl_all = accp.tile([P, CH], F32)
            G2 = accp.tile([P, 8 * CH], F32)  # (t,v,a,s) at ii, ii+1
            G1r = G1[:].rearrange("p (c k) -> p c k", k=4)
            G2r = G2[:].rearrange("p (c k) -> p c k", k=8)

            rt_tiles = {}

            def heavy(c):
                """stream + dist2 + argmin for chunk c (dense big-op section)"""
                rows = slice(c * P, (c + 1) * P)
                xym_t = stream.tile([P, 2, T], F32, tag="xym", name="xym_t")
                nc.sync.dma_start(out=xym_t[:], in_=xym_d[rows])
                rt_t = stream.tile([P, T], F32, tag="rt", bufs=CH, name="rt_t")
                nc.sync.dma_start(out=rt_t[:], in_=rt_d[rows])
                rt_tiles[c] = rt_t

                dx2 = work.tile([P, T], F32, tag="dx2", name="dx2")
                nc.scalar.activation(
                    dx2[:], xym_t[:, 0, :], AF.Square,
                    bias=vec[:, c : c + 1], scale=1.0,
                )
                dy2 = work.tile([P, T], F32, tag="dy2", name="dy2")
                nc.scalar.activation(
                    dy2[:], xym_t[:, 1, :], AF.Square,
                    bias=vec[:, 4 + c : 5 + c], scale=1.0,
                )
                dist2 = work.tile([P, T], F32, tag="dist2", name="dist2")
                nc.vector.tensor_tensor(
                    out=dist2[:], in0=dx2[:], in1=dy2[:], op=OP.add
                )
                minv = small.tile([P, 1], F32, tag="minv", name="minv")
                nc.vector.tensor_reduce(
                    out=minv[:], in_=dist2[:], axis=mybir.AxisListType.X, op=OP.min
                )
                nc.vector.max_index(
                    idx_all[:, 8 * c : 8 * c + 8],
                    minv[:, 0:1].to_broadcast([P, 8]),
                    dist2[:],
                )

            def tail(c):
                """gather1 -> t_cl -> sign count -> gather2 for chunk c
                (latency-bound small ops; hidden under the next chunk's heavy)"""
                off1 = small.tile([P, 1], U32, tag="off1", name="off1")
                nc.vector.tensor_tensor(
                    out=off1[:], in0=idx_all[:, 8 * c : 8 * c + 1],
                    in1=rbcu[:, c : c + 1], op=OP.add,
                )
                nc.gpsimd.indirect_dma_start(
                    out=G1[:, 4 * c : 4 * c + 4],
                    out_offset=None,
                    in_=tvas_d,
                    in_offset=IndirectOffsetOnAxis(ap=off1[:, 0:1], axis=0),
                )
                # t_cl = min(t_m + 0.8, t_max)  [t_m >= 0 so the max(.,0) is dead]
                nc.vector.tensor_scalar(
                    out=tcl_all[:, c : c + 1],
                    in0=G1[:, 4 * c : 4 * c + 1],
                    scalar1=PREVIEW_WINDOW,
                    scalar2=vec[:, 12 + c : 13 + c],
                    op0=OP.add,
                    op1=OP.min,
                )
                # S = sum_j sign(t_cl - t_j)
                cntscr = work.tile([P, T], F32, tag="dx2", name="cntscr")
                nc.scalar.activation(
                    cntscr[:], rt_tiles[c][:], AF.Sign,
                    bias=tcl_all[:, c : c + 1], scale=-1.0,
                    accum_out=S_all[:, c : c + 1],
                )
                # cnt = f*(S/2-1024) + (S/2+1024), f = (t_cl >= t_max)
                dlt = small.tile([P, 1], F32, tag="dlt", name="dlt")
                nc.vector.tensor_scalar(
                    out=dlt[:], in0=S_all[:, c : c + 1], scalar1=0.5,
                    scalar2=-float(T // 2), op0=OP.mult, op1=OP.add,
                )
                a_t = small.tile([P, 1], F32, tag="a_t", name="a_t")
                nc.vector.tensor_scalar(
                    out=a_t[:], in0=S_all[:, c : c + 1], scalar1=0.5,
                    scalar2=float(T // 2), op0=OP.mult, op1=OP.add,
                )
                e_t = small.tile([P, 1], F32, tag="e_t", name="e_t")
                nc.vector.tensor_scalar(
                    out=e_t[:], in0=tcl_all[:, c : c + 1],
                    scalar1=vec[:, 12 + c : 13 + c], scalar2=dlt[:, 0:1],
                    op0=OP.is_ge, op1=OP.mult,
                )
                cnt = small.tile([P, 1], F32, tag="cnt", name="cnt")
                nc.vector.tensor_tensor(
                    out=cnt[:], in0=e_t[:], in1=a_t[:], op=OP.add
                )
                # off2 = (cnt - 1) + rowbase, via rbm1; cnt >= 1 always
                cnti = small.tile([P, 1], I32, tag="cnti", name="cnti")
                nc.vector.tensor_copy(cnti[:], cnt[:])
                off2 = small.tile([P, 1], I32, tag="off2", name="off2")
                nc.vector.tensor_tensor(
                    out=off2[:], in0=cnti[:], in1=rbm1[:, c : c + 1], op=OP.add
                )
                nc.gpsimd.indirect_dma_start(
                    out=G2[:, 8 * c : 8 * c + 8],
                    out_offset=None,
                    in_=tvas_d,
                    in_offset=IndirectOffsetOnAxis(ap=off2[:, 0:1], axis=0),
                )

            # software-pipelined emission: chunk c's heavy ops are issued
            # before chunk c-1's latency-bound tail so per-engine in-order
            # streams never stall on gather round-trips
            heavy(0)
            heavy(1)
            tail(0)
            heavy(2)
            tail(1)
            heavy(3)
            tail(2)
            tail(3)

            # ---- phase F: batched frac + interpolation + PID ----
            # frac = clip((t_cl - t0) / (t1 - t0), 0, 1)   [t1-t0 ~ 0.1 > 0]
            t0v = G2r[:, :, 0]
            den = small.tile([P, CH], F32, tag="den")
            nc.vector.tensor_tensor(
                out=den[:], in0=G2r[:, :, 4], in1=t0v, op=OP.subtract
            )
            rec = small.tile([P, CH], F32, tag="rec")
            nc.vector.reciprocal(rec[:], den[:])
            num = small.tile([P, CH], F32, tag="num")
            nc.vector.tensor_tensor(
                out=num[:], in0=tcl_all[:], in1=t0v, op=OP.subtract
            )
            fr = small.tile([P, CH], F32, tag="fr")
            nc.vector.tensor_tensor(out=fr[:], in0=num[:], in1=rec[:], op=OP.mult)
            frac_all = small.tile([P, CH], F32, tag="frac_all")
            nc.vector.tensor_scalar(
                out=frac_all[:], in0=fr[:], scalar1=0.0, scalar2=1.0,
                op0=OP.max, op1=OP.min,
            )

            Dall = accp.tile([P, 4 * CH], F32)
            Dr = Dall[:].rearrange("p (c k) -> p c k", k=4)
            nc.vector.tensor_tensor(
                out=Dr, in0=G2r[:, :, 4:8], in1=G2r[:, :, 0:4], op=OP.subtract
            )
            Pall = accp.tile([P, 4 * CH], F32)
            for c in range(CH):
                nc.vector.tensor_scalar(
                    out=Pall[:, 4 * c : 4 * c + 4],
                    in0=Dall[:, 4 * c : 4 * c + 4],
                    scalar1=frac_all[:, c : c + 1],
                    scalar2=None,
                    op0=OP.mult,
                )
            Iall = accp.tile([P, 4 * CH], F32)
            Ir = Iall[:].rearrange("p (c k) -> p c k", k=4)
            Pr = Pall[:].rearrange("p (c k) -> p c k", k=4)
            nc.vector.tensor_tensor(
                out=Ir, in0=Pr, in1=G2r[:, :, 0:4], op=OP.add
            )

            s_m = G1r[:, :, 3]  # [P, CH] strided view
            v_p = Ir[:, :, 1]
            a_p = Ir[:, :, 2]
            s_p = Ir[:, :, 3]

            def pt(tag):
                return small.tile([P, CH], F32, tag=tag, name=tag)

            # station PI: station_err = 5*tanh((s_p-s_m)/5), folded as th*5
            serr0 = pt("serr0")
            nc.vector.tensor_tensor(out=serr0[:], in0=s_p, in1=s_m, op=OP.subtract)
            th = pt("th")
            nc.scalar.activation(
                th[:], serr0[:], AF.Tanh, scale=float(1.0 / STATION_ERR_LIM)
            )
            t1a = pt("t1a")  # station_err*DT = th*0.1
            nc.scalar.activation(t1a[:], th[:], AF.Identity, scale=0.1)
            ints0 = pt("ints0")
            nc.vector.tensor_tensor(
                out=ints0[:], in0=t1a[:], in1=vec[:, 16:20], op=OP.add
            )
            ints = pt("ints")
            nc.vector.tensor_scalar(
                out=ints[:], in0=ints0[:], scalar1=-INTEGRATOR_SAT,
                scalar2=INTEGRATOR_SAT, op0=OP.max, op1=OP.min,
            )
            so1 = pt("so1")  # station_kp*station_err = th*(5*station_kp)
            nc.scalar.activation(so1[:], th[:], AF.Identity, scale=vec[:, 24:25])
            so2 = pt("so2")
            nc.scalar.activation(so2[:], ints[:], AF.Identity, scale=vec[:, 25:26])
            soff = pt("soff")
            nc.vector.tensor_tensor(out=soff[:], in0=so1[:], in1=so2[:], op=OP.add)

            # speed PI: speed_err = 3*tanh(ve1/3) folded as th2*3
            ve0 = pt("ve0")
            nc.vector.tensor_tensor(out=ve0[:], in0=v_p, in1=soff[:], op=OP.add)
            ve1 = pt("ve1")
            nc.vector.tensor_tensor(
                out=ve1[:], in0=ve0[:], in1=vec[:, 8:12], op=OP.subtract
            )
            th2 = pt("th2")
            nc.scalar.activation(
                th2[:], ve1[:], AF.Tanh, scale=float(1.0 / SPEED_INPUT_LIM)
            )
            t2a = pt("t2a")  # speed_err*DT = th2*0.06
            nc.scalar.activation(t2a[:], th2[:], AF.Identity, scale=0.06)
            insp0 = pt("insp0")
            nc.vector.tensor_tensor(
                out=insp0[:], in0=t2a[:], in1=vec[:, 20:24], op=OP.add
            )
            insp = pt("insp")
            nc.vector.tensor_scalar(
                out=insp[:], in0=insp0[:], scalar1=-INTEGRATOR_SAT,
                scalar2=INTEGRATOR_SAT, op0=OP.max, op1=OP.min,
            )
            w = pt("w")
            nc.scalar.activation(
                w[:], vec[:, 8:12], AF.Sigmoid, bias=vec[:, 30:31], scale=2.0
            )
            kp3 = pt("kp3")  # 3*kp = w*dkp3 + lokp3
            nc.scalar.activation(
                kp3[:], w[:], AF.Identity, scale=vec[:, 28:29], bias=vec[:, 26:27]
            )
            ki = pt("ki")
            nc.scalar.activation(
                ki[:], w[:], AF.Identity, scale=vec[:, 29:30], bias=vec[:, 27:28]
            )
            p1 = pt("p1")  # kp*speed_err = kp3*th2
            nc.vector.tensor_tensor(out=p1[:], in0=kp3[:], in1=th2[:], op=OP.mult)
            p2 = pt("p2")
            nc.vector.tensor_tensor(out=p2[:], in0=ki[:], in1=insp[:], op=OP.mult)
            p3 = pt("p3")
            nc.vector.tensor_tensor(out=p3[:], in0=p1[:], in1=p2[:], op=OP.add)
            p4 = pt("p4")
            nc.vector.tensor_tensor(out=p4[:], in0=p3[:], in1=a_p, op=OP.add)
            accf = pt("accf")
            nc.vector.tensor_scalar(
                out=accf[:], in0=p4[:], scalar1=ACC_MIN, scalar2=ACC_MAX,
                op0=OP.max, op1=OP.min,
            )
            nc.sync.dma_start(out=out_d, in_=accf[:])

    nc.compile()
    _CACHE["nc"] = nc
    return nc


def _prepare_in_maps(inputs):
    def f(name):
        return np.ascontiguousarray(np.asarray(inputs[name], dtype=np.float32))

    rx = f("ref_x")
    ry = f("ref_y")
    rt = f("ref_t")
    valid = f("valid_mask")
    ym = np.where(valid > 0.5, ry, np.float32(MASK_BIG)).astype(np.float32)
    xym = np.stack([rx, ym], axis=1)  # [B, 2, T] contiguous
    tvas = np.stack(
        [rt, f("ref_v"), f("ref_a"), f("ref_s")], axis=2
    )  # [B, T, 4] contiguous

    xs = f("x")
    ys = f("y")
    vs = f("v")
    tmax = f("t_max")
    ist = f("integral_station")
    isp = f("integral_speed")

    sk = np.float32(np.asarray(inputs["station_kp"]))
    si = np.float32(np.asarray(inputs["station_ki"]))
    lkp = np.float32(np.asarray(inputs["low_speed_kp"]))
    lki = np.float32(np.asarray(inputs["low_speed_ki"]))
    hkp = np.float32(np.asarray(inputs["high_speed_kp"]))
    hki = np.float32(np.asarray(inputs["high_speed_ki"]))
    sw = np.float32(np.asarray(inputs["switch_speed"]))

    in_maps = []
    for core in range(NCORES):
        base = core * RPC
        sl = slice(base, base + RPC)
        vec = np.zeros((P, 32), np.float32)
        for c in range(CH):
            rows = slice(base + c * P, base + (c + 1) * P)
            vec[:, 0 + c] = -xs[rows]
            vec[:, 4 + c] = -ys[rows]
            vec[:, 8 + c] = vs[rows]
            vec[:, 12 + c] = tmax[rows]
            vec[:, 16 + c] = ist[rows]
            vec[:, 20 + c] = isp[rows]
        vec[:, 24] = np.float32(5.0) * sk
        vec[:, 25] = si
        vec[:, 26] = np.float32(3.0) * lkp
        vec[:, 27] = lki
        vec[:, 28] = np.float32(3.0) * (hkp - lkp)
        vec[:, 29] = hki - lki
        vec[:, 30] = np.float32(-2.0) * sw
        in_maps.append(
            {
                "xym": xym[sl],
                "rt": np.ascontiguousarray(rt[sl]),
                "tvas": tvas[sl].reshape(RPC * T, 4),
                "vec": vec,
            }
        )
    return in_maps


def _assemble(results):
    out = np.empty(B, np.float32)
    for core in range(NCORES):
        oc = np.asarray(results[core]["out"], np.float32)  # [P, CH]
        out[core * RPC : (core + 1) * RPC] = oc.T.reshape(RPC)
    return out


def kernel(**inputs):
    nc = _build_program()
    in_maps = _prepare_in_maps(inputs)
    res = run_bass_kernel_spmd(nc, in_maps, core_ids=list(range(NCORES)))
    return _assemble(res.results)


def kernel_traced(inputs, **kwargs):
    """For test.py: same as kernel() but returns (output, BassKernelResults)."""
    nc = _build_program()
    in_maps = _prepare_in_maps(inputs)
    res = run_bass_kernel_spmd(
        nc, in_maps, core_ids=list(range(NCORES)), trace=True, **kwargs
    )
    return _assemble(res.results), res


# revision 36
# speedup vs baseline: 1.1106x; 1.0159x over previous
"""Trainium2 Bass kernel for BatchedLonCtrl (retrieval_knn).

Contract: kernel(**inputs) takes the FULL unsharded inputs (as produced by
setup_inputs()) and returns the FULL [B] float32 output. Internally the batch
dim is sharded across 8 NeuronCores (pure data parallel), the Bass program is
compiled once and run via run_bass_kernel_spmd.

Device algorithm per core (512 rows = 4 chunks x 128 partitions):
  1. stream ref_x, ref_y(masked), ref_t row-chunks into SBUF
  2. dist2 = (rx-x)^2 + (ry-y)^2 via ACT Square + DVE add / min-reduce
     (valid_mask is pre-folded into ref_y on host: invalid -> 1e9 -> dist2 ~1e18)
  3. argmin index via DVE max_index (value matcher) on the min value
  4. gather (t,v,a,s)[idx] via indirect DMA from a host-packed [T,4] interleave
  5. searchsorted(ref_t, t_cl) as a count of (ref_t < t_cl); either a DVE
     is_lt+accum pass or an ACT Sign+accum pass with an exact fixup
  6. gather (t,v,a,s)[ii], (t,v,a,s)[ii+1] in one 8-wide indirect DMA
  7. linear interp + station/speed PID + clamps, batched [128,4] per core
"""

import numpy as np

try:
    import concourse.bass as bass
except ImportError:  # environment provides the repo at /opt/trn_rl_repo
    import sys

    sys.path.insert(0, "/opt/trn_rl_repo")
    import concourse.bass as bass

import concourse.bacc as bacc
import concourse.tile as tile
from concourse import mybir
from concourse.bass import IndirectOffsetOnAxis
from concourse.bass_utils import run_bass_kernel_spmd

F32 = mybir.dt.float32
I32 = mybir.dt.int32
U32 = mybir.dt.uint32
AF = mybir.ActivationFunctionType
OP = mybir.AluOpType

B, T = 4096, 2048
NCORES = 8
RPC = B // NCORES  # rows per core = 512
P = 128
CH = RPC // P  # chunks per core = 4

DT = 0.02
PREVIEW_WINDOW = 0.8
STATION_ERR_LIM = 5.0
SPEED_INPUT_LIM = 3.0
INTEGRATOR_SAT = 5.0
ACC_MIN, ACC_MAX = -4.0, 2.0
MASK_BIG = 1.0e9  # invalid ref_y replacement; dist2 becomes ~1e18 >> any valid

# feature flags (validated per-op on HW)
USE_ICOPY_TM = True  # t_m via gpsimd.indirect_copy from the SBUF rt tile
# (instead of waiting on the gather1 indirect-DMA round trip)

_CACHE = {}


def _build_program():
    if "nc" in _CACHE:
        return _CACHE["nc"]

    nc = bacc.Bacc(
        "TRN2", target_bir_lowering=False, debug=False, enable_asserts=False
    )

    xym_d = nc.dram_tensor("xym", [RPC, 2, T], F32, kind="ExternalInput").ap()
    rt_d = nc.dram_tensor("rt", [RPC, T], F32, kind="ExternalInput").ap()
    tvas_d = nc.dram_tensor("tvas", [RPC * T, 4], F32, kind="ExternalInput").ap()
    vec_d = nc.dram_tensor("vec", [P, 48], F32, kind="ExternalInput").ap()
    out_d = nc.dram_tensor("out", [P, CH], F32, kind="ExternalOutput").ap()

    # vec columns:
    #  0: 4   -x per chunk          4: 8   -y per chunk      8:12  v per chunk
    # 12:16   t_max per chunk      16:20   integral_station  20:24 integral_speed
    # 24 kp5=5*station_kp  25 station_ki  26 lokp3=3*low_kp  27 low_ki
    # 28 dkp3=3*(high_kp-low_kp)  29 dki=high_ki-low_ki  30 -2*switch_speed
    # 32:48 identity-16 mask for the indirect_copy diagonal extract

    with tile.TileContext(nc) as tc:
        from contextlib import ExitStack

        with ExitStack() as ctx:
            singles = ctx.enter_context(tc.tile_pool(name="singles", bufs=1))
            stream = ctx.enter_context(tc.tile_pool(name="stream", bufs=2))
            work = ctx.enter_context(tc.tile_pool(name="work", bufs=2))
            small = ctx.enter_context(tc.tile_pool(name="small", bufs=2))
            accp = ctx.enter_context(tc.tile_pool(name="accp", bufs=1))

            vec = singles.tile([P, 48], F32)
            nc.sync.dma_start(out=vec[:], in_=vec_d)

            # rbcu[:, c] = p*T + c*128*T  (tvas row base for gather1, int32)
            # rbm1[:, c] = p*T + c*128*T - 1  (base with the ii=cnt-1 folded in)
            rbcu = singles.tile([P, CH], U32)
            rbm1 = singles.tile([P, CH], I32)
            for c in range(CH):
                nc.gpsimd.iota(
                    rbcu[:, c : c + 1], pattern=[[1, 1]],
                    base=c * P * T, channel_multiplier=T,
                )
                nc.gpsimd.iota(
                    rbm1[:, c : c + 1], pattern=[[1, 1]],
                    base=c * P * T - 1, channel_multiplier=T,
                )

            # per-core accumulators
            idx_all = accp.tile([P, 8 * CH], U32)  # FIND_INDEX8 outputs
            off1_all = accp.tile([P, CH], U32)
            G1 = accp.tile([P, 4 * CH], F32)  # (t,v,a,s) at idx, per chunk
            S_all = accp.tile([P, CH], F32)
            tcl_all = accp.tile([P, CH], F32)
            G2 = accp.tile([P, 8 * CH], F32)  # (t,v,a,s) at ii, ii+1
            G1r = G1[:].rearrange("p (c k) -> p c k", k=4)
            G2r = G2[:].rearrange("p (c k) -> p c k", k=8)

            rt_tiles = {}

            def heavy(c):
                """stream + dist2 + argmin for chunk c (dense big-op section)"""
                rows = slice(c * P, (c + 1) * P)
                xym_t = stream.tile([P, 2, T], F32, tag="xym", bufs=3, name="xym_t")
                nc.sync.dma_start(out=xym_t[:], in_=xym_d[rows])
                rt_t = stream.tile([P, T], F32, tag="rt", bufs=CH, name="rt_t")
                nc.sync.dma_start(out=rt_t[:], in_=rt_d[rows])
                rt_tiles[c] = rt_t

                dx2 = work.tile([P, T], F32, tag="dx2", bufs=3, name="dx2")
                nc.scalar.activation(
                    dx2[:], xym_t[:, 0, :], AF.Square,
                    bias=vec[:, c : c + 1], scale=1.0,
                )
                dy2 = work.tile([P, T], F32, tag="dy2", bufs=3, name="dy2")
                nc.scalar.activation(
                    dy2[:], xym_t[:, 1, :], AF.Square,
                    bias=vec[:, 4 + c : 5 + c], scale=1.0,
                )
                dist2 = work.tile([P, T], F32, tag="dist2", bufs=3, name="dist2")
                nc.vector.tensor_tensor(
                    out=dist2[:], in0=dx2[:], in1=dy2[:], op=OP.add
                )
                minv = small.tile([P, 1], F32, tag="minv", name="minv")
                nc.vector.tensor_reduce(
                    out=minv[:], in_=dist2[:], axis=mybir.AxisListType.X, op=OP.min
                )
                nc.vector.max_index(
                    idx_all[:, 8 * c : 8 * c + 8],
                    minv[:, 0:1].to_broadcast([P, 8]),
                    dist2[:],
                )
                # gather1 offset, then issue gather1 immediately: its ~5us
                # indirect-DMA latency hides under the next two chunks' heavy
                # ops (tail(c) is emitted two heavies later)
                nc.vector.tensor_tensor(
                    out=off1_all[:, c : c + 1], in0=idx_all[:, 8 * c : 8 * c + 1],
                    in1=rbcu[:, c : c + 1], op=OP.add,
                )
                nc.gpsimd.indirect_dma_start(
                    out=G1[:, 4 * c : 4 * c + 4],
                    out_offset=None,
                    in_=tvas_d,
                    in_offset=IndirectOffsetOnAxis(ap=off1_all[:, c : c + 1], axis=0),
                )

            def tail(c):
                """t_cl -> sign count -> gather2 for chunk c; G1[c] is already
                resident (issued in heavy(c)), so nothing here waits on a
                same-chunk gather round trip"""
                # t_cl = min(t_m + 0.8, t_max)  [t_m >= 0 so the max(.,0) is dead]
                nc.vector.tensor_scalar(
                    out=tcl_all[:, c : c + 1],
                    in0=G1[:, 4 * c : 4 * c + 1],
                    scalar1=PREVIEW_WINDOW,
                    scalar2=vec[:, 12 + c : 13 + c],
                    op0=OP.add,
                    op1=OP.min,
                )
                # S = sum_j sign(t_cl - t_j)
                cntscr = work.tile([P, T], F32, tag="dx2", bufs=3, name="cntscr")
                nc.scalar.activation(
                    cntscr[:], rt_tiles[c][:], AF.Sign,
                    bias=tcl_all[:, c : c + 1], scale=-1.0,
                    accum_out=S_all[:, c : c + 1],
                )
                # cnt = f*(S/2-1024) + (S/2+1024), f = (t_cl >= t_max)
                dlt = small.tile([P, 1], F32, tag="dlt", name="dlt")
                nc.vector.tensor_scalar(
                    out=dlt[:], in0=S_all[:, c : c + 1], scalar1=0.5,
                    scalar2=-float(T // 2), op0=OP.mult, op1=OP.add,
                )
                a_t = small.tile([P, 1], F32, tag="a_t", name="a_t")
                nc.vector.tensor_scalar(
                    out=a_t[:], in0=S_all[:, c : c + 1], scalar1=0.5,
                    scalar2=float(T // 2), op0=OP.mult, op1=OP.add,
                )
                e_t = small.tile([P, 1], F32, tag="e_t", name="e_t")
                nc.vector.tensor_scalar(
                    out=e_t[:], in0=tcl_all[:, c : c + 1],
                    scalar1=vec[:, 12 + c : 13 + c], scalar2=dlt[:, 0:1],
                    op0=OP.is_ge, op1=OP.mult,
                )
                cnt = small.tile([P, 1], F32, tag="cnt", name="cnt")
                nc.vector.tensor_tensor(
                    out=cnt[:], in0=e_t[:], in1=a_t[:], op=OP.add
                )
                # off2 = (cnt - 1) + rowbase, via rbm1; cnt >= 1 always
                cnti = small.tile([P, 1], I32, tag="cnti", name="cnti")
                nc.vector.tensor_copy(cnti[:], cnt[:])
                off2 = small.tile([P, 1], I32, tag="off2", name="off2")
                nc.vector.tensor_tensor(
                    out=off2[:], in0=cnti[:], in1=rbm1[:, c : c + 1], op=OP.add
                )
                nc.gpsimd.indirect_dma_start(
                    out=G2[:, 8 * c : 8 * c + 8],
                    out_offset=None,
                    in_=tvas_d,
                    in_offset=IndirectOffsetOnAxis(ap=off2[:, 0:1], axis=0),
                )

            # software-pipelined emission (depth 2): chunk c's tail is issued
            # two heavies later so per-engine in-order streams never stall on
            # gather round-trips
            heavy(0)
            heavy(1)
            heavy(2)
            tail(0)
            heavy(3)
            tail(1)
            tail(2)
            tail(3)

            # ---- phase F: batched frac + interpolation + PID ----
            # frac = clip((t_cl - t0) / (t1 - t0), 0, 1)   [t1-t0 ~ 0.1 > 0]
            t0v = G2r[:, :, 0]
            den = small.tile([P, CH], F32, tag="den")
            nc.vector.tensor_tensor(
                out=den[:], in0=G2r[:, :, 4], in1=t0v, op=OP.subtract
            )
            rec = small.tile([P, CH], F32, tag="rec")
            nc.vector.reciprocal(rec[:], den[:])
            num = small.tile([P, CH], F32, tag="num")
            nc.vector.tensor_tensor(
                out=num[:], in0=tcl_all[:], in1=t0v, op=OP.subtract
            )
            fr = small.tile([P, CH], F32, tag="fr")
            nc.vector.tensor_tensor(out=fr[:], in0=num[:], in1=rec[:], op=OP.mult)
            frac_all = small.tile([P, CH], F32, tag="frac_all")
            nc.vector.tensor_scalar(
                out=frac_all[:], in0=fr[:], scalar1=0.0, scalar2=1.0,
                op0=OP.max, op1=OP.min,
            )

            Dall = accp.tile([P, 4 * CH], F32)
            Dr = Dall[:].rearrange("p (c k) -> p c k", k=4)
            nc.vector.tensor_tensor(
                out=Dr, in0=G2r[:, :, 4:8], in1=G2r[:, :, 0:4], op=OP.subtract
            )
            Pall = accp.tile([P, 4 * CH], F32)
            for c in range(CH):
                nc.vector.tensor_scalar(
                    out=Pall[:, 4 * c : 4 * c + 4],
                    in0=Dall[:, 4 * c : 4 * c + 4],
                    scalar1=frac_all[:, c : c + 1],
                    scalar2=None,
                    op0=OP.mult,
                )
            Iall = accp.tile([P, 4 * CH], F32)
            Ir = Iall[:].rearrange("p (c k) -> p c k", k=4)
            Pr = Pall[:].rearrange("p (c k) -> p c k", k=4)
            nc.vector.tensor_tensor(
                out=Ir, in0=Pr, in1=G2r[:, :, 0:4], op=OP.add
            )

            s_m = G1r[:, :, 3]  # [P, CH] strided view
            v_p = Ir[:, :, 1]
            a_p = Ir[:, :, 2]
            s_p = Ir[:, :, 3]

            def pt(tag):
                return small.tile([P, CH], F32, tag=tag, name=tag)

            # station PI: station_err = 5*tanh((s_p-s_m)/5), folded as th*5
            serr0 = pt("serr0")
            nc.vector.tensor_tensor(out=serr0[:], in0=s_p, in1=s_m, op=OP.subtract)
            th = pt("th")
            nc.scalar.activation(
                th[:], serr0[:], AF.Tanh, scale=float(1.0 / STATION_ERR_LIM)
            )
            t1a = pt("t1a")  # station_err*DT = th*0.1
            nc.scalar.activation(t1a[:], th[:], AF.Identity, scale=0.1)
            ints0 = pt("ints0")
            nc.vector.tensor_tensor(
                out=ints0[:], in0=t1a[:], in1=vec[:, 16:20], op=OP.add
            )
            ints = pt("ints")
            nc.vector.tensor_scalar(
                out=ints[:], in0=ints0[:], scalar1=-INTEGRATOR_SAT,
                scalar2=INTEGRATOR_SAT, op0=OP.max, op1=OP.min,
            )
            so1 = pt("so1")  # station_kp*station_err = th*(5*station_kp)
            nc.scalar.activation(so1[:], th[:], AF.Identity, scale=vec[:, 24:25])
            so2 = pt("so2")
            nc.scalar.activation(so2[:], ints[:], AF.Identity, scale=vec[:, 25:26])
            soff = pt("soff")
            nc.vector.tensor_tensor(out=soff[:], in0=so1[:], in1=so2[:], op=OP.add)

            # speed PI: speed_err = 3*tanh(ve1/3) folded as th2*3
            ve0 = pt("ve0")
            nc.vector.tensor_tensor(out=ve0[:], in0=v_p, in1=soff[:], op=OP.add)
            ve1 = pt("ve1")
            nc.vector.tensor_tensor(
                out=ve1[:], in0=ve0[:], in1=vec[:, 8:12], op=OP.subtract
            )
            th2 = pt("th2")
            nc.scalar.activation(
                th2[:], ve1[:], AF.Tanh, scale=float(1.0 / SPEED_INPUT_LIM)
            )
            t2a = pt("t2a")  # speed_err*DT = th2*0.06
            nc.scalar.activation(t2a[:], th2[:], AF.Identity, scale=0.06)
            insp0 = pt("insp0")
            nc.vector.tensor_tensor(
                out=insp0[:], in0=t2a[:], in1=vec[:, 20:24], op=OP.add
            )
            insp = pt("insp")
            nc.vector.tensor_scalar(
                out=insp[:], in0=insp0[:], scalar1=-INTEGRATOR_SAT,
                scalar2=INTEGRATOR_SAT, op0=OP.max, op1=OP.min,
            )
            w = pt("w")
            nc.scalar.activation(
                w[:], vec[:, 8:12], AF.Sigmoid, bias=vec[:, 30:31], scale=2.0
            )
            kp3 = pt("kp3")  # 3*kp = w*dkp3 + lokp3
            nc.scalar.activation(
                kp3[:], w[:], AF.Identity, scale=vec[:, 28:29], bias=vec[:, 26:27]
            )
            ki = pt("ki")
            nc.scalar.activation(
                ki[:], w[:], AF.Identity, scale=vec[:, 29:30], bias=vec[:, 27:28]
            )
            p1 = pt("p1")  # kp*speed_err = kp3*th2
            nc.vector.tensor_tensor(out=p1[:], in0=kp3[:], in1=th2[:], op=OP.mult)
            p2 = pt("p2")
            nc.vector.tensor_tensor(out=p2[:], in0=ki[:], in1=insp[:], op=OP.mult)
            p3 = pt("p3")
            nc.vector.tensor_tensor(out=p3[:], in0=p1[:], in1=p2[:], op=OP.add)
            p4 = pt("p4")
            nc.vector.tensor_tensor(out=p4[:], in0=p3[:], in1=a_p, op=OP.add)
            accf = pt("accf")
            nc.vector.tensor_scalar(
                out=accf[:], in0=p4[:], scalar1=ACC_MIN, scalar2=ACC_MAX,
                op0=OP.max, op1=OP.min,
            )
            nc.sync.dma_start(out=out_d, in_=accf[:])

    nc.compile()
    _CACHE["nc"] = nc
    return nc


def _prepare_in_maps(inputs):
    def f(name):
        return np.ascontiguousarray(np.asarray(inputs[name], dtype=np.float32))

    rx = f("ref_x")
    ry = f("ref_y")
    rt = f("ref_t")
    valid = f("valid_mask")
    ym = np.where(valid > 0.5, ry, np.float32(MASK_BIG)).astype(np.float32)
    xym = np.stack([rx, ym], axis=1)  # [B, 2, T] contiguous
    tvas = np.stack(
        [rt, f("ref_v"), f("ref_a"), f("ref_s")], axis=2
    )  # [B, T, 4] contiguous

    xs = f("x")
    ys = f("y")
    vs = f("v")
    tmax = f("t_max")
    ist = f("integral_station")
    isp = f("integral_speed")

    sk = np.float32(np.asarray(inputs["station_kp"]))
    si = np.float32(np.asarray(inputs["station_ki"]))
    lkp = np.float32(np.asarray(inputs["low_speed_kp"]))
    lki = np.float32(np.asarray(inputs["low_speed_ki"]))
    hkp = np.float32(np.asarray(inputs["high_speed_kp"]))
    hki = np.float32(np.asarray(inputs["high_speed_ki"]))
    sw = np.float32(np.asarray(inputs["switch_speed"]))

    in_maps = []
    for core in range(NCORES):
        base = core * RPC
        sl = slice(base, base + RPC)
        vec = np.zeros((P, 48), np.float32)
        for c in range(CH):
            rows = slice(base + c * P, base + (c + 1) * P)
            vec[:, 0 + c] = -xs[rows]
            vec[:, 4 + c] = -ys[rows]
            vec[:, 8 + c] = vs[rows]
            vec[:, 12 + c] = tmax[rows]
            vec[:, 16 + c] = ist[rows]
            vec[:, 20 + c] = isp[rows]
        vec[:, 24] = np.float32(5.0) * sk
        vec[:, 25] = si
        vec[:, 26] = np.float32(3.0) * lkp
        vec[:, 27] = lki
        vec[:, 28] = np.float32(3.0) * (hkp - lkp)
        vec[:, 29] = hki - lki
        vec[:, 30] = np.float32(-2.0) * sw
        vec[np.arange(P), 32 + (np.arange(P) % 16)] = 1.0
        in_maps.append(
            {
                "xym": xym[sl],
                "rt": np.ascontiguousarray(rt[sl]),
                "tvas": tvas[sl].reshape(RPC * T, 4),
                "vec": vec,
            }
        )
    return in_maps


def _assemble(results):
    out = np.empty(B, np.float32)
    for core in range(NCORES):
        oc = np.asarray(results[core]["out"], np.float32)  # [P, CH]
        out[core * RPC : (core + 1) * RPC] = oc.T.reshape(RPC)
    return out


def kernel(**inputs):
    nc = _build_program()
    in_maps = _prepare_in_maps(inputs)
    res = run_bass_kernel_spmd(nc, in_maps, core_ids=list(range(NCORES)))
    return _assemble(res.results)


def kernel_traced(inputs, **kwargs):
    """For test.py: same as kernel() but returns (output, BassKernelResults)."""
    nc = _build_program()
    in_maps = _prepare_in_maps(inputs)
    res = run_bass_kernel_spmd(
        nc, in_maps, core_ids=list(range(NCORES)), trace=True, **kwargs
    )
    return _assemble(res.results), res


# revision 39
# speedup vs baseline: 1.1358x; 1.0226x over previous
"""Trainium2 Bass kernel for BatchedLonCtrl (retrieval_knn).

Contract: kernel(**inputs) takes the FULL unsharded inputs (as produced by
setup_inputs()) and returns the FULL [B] float32 output. Internally the batch
dim is sharded across 8 NeuronCores (pure data parallel), the Bass program is
compiled once and run via run_bass_kernel_spmd.

Device algorithm per core (512 rows = 4 chunks x 128 partitions):
  1. stream ref_x, ref_y(masked), ref_t row-chunks into SBUF
  2. dist2 = (rx-x)^2 + (ry-y)^2 via ACT Square + DVE add / min-reduce
     (valid_mask is pre-folded into ref_y on host: invalid -> 1e9 -> dist2 ~1e18)
  3. argmin index via DVE max_index (value matcher) on the min value
  4. gather (t,v,a,s)[idx] via indirect DMA from a host-packed [T,4] interleave
  5. searchsorted(ref_t, t_cl) as a count of (ref_t < t_cl); either a DVE
     is_lt+accum pass or an ACT Sign+accum pass with an exact fixup
  6. gather (t,v,a,s)[ii], (t,v,a,s)[ii+1] in one 8-wide indirect DMA
  7. linear interp + station/speed PID + clamps, batched [128,4] per core
"""

import numpy as np

try:
    import concourse.bass as bass
except ImportError:  # environment provides the repo at /opt/trn_rl_repo
    import sys

    sys.path.insert(0, "/opt/trn_rl_repo")
    import concourse.bass as bass

import concourse.bacc as bacc
import concourse.tile as tile
from concourse import mybir
from concourse.bass import IndirectOffsetOnAxis
from concourse.bass_utils import run_bass_kernel_spmd

F32 = mybir.dt.float32
I32 = mybir.dt.int32
U32 = mybir.dt.uint32
AF = mybir.ActivationFunctionType
OP = mybir.AluOpType

B, T = 4096, 2048
NCORES = 8
RPC = B // NCORES  # rows per core = 512
P = 128
CH = RPC // P  # chunks per core = 4

DT = 0.02
PREVIEW_WINDOW = 0.8
STATION_ERR_LIM = 5.0
SPEED_INPUT_LIM = 3.0
INTEGRATOR_SAT = 5.0
ACC_MIN, ACC_MAX = -4.0, 2.0
MASK_BIG = 1.0e9  # invalid ref_y replacement; dist2 becomes ~1e18 >> any valid

# feature flags (validated per-op on HW)
USE_ICOPY_TM = True  # t_m via gpsimd.indirect_copy from the SBUF rt tile
# (instead of waiting on the gather1 indirect-DMA round trip)

_CACHE = {}


def _build_program():
    if "nc" in _CACHE:
        return _CACHE["nc"]

    nc = bacc.Bacc(
        "TRN2", target_bir_lowering=False, debug=False, enable_asserts=False
    )

    xym_d = nc.dram_tensor("xym", [RPC, 2, T], F32, kind="ExternalInput").ap()
    rt_d = nc.dram_tensor("rt", [RPC, T], F32, kind="ExternalInput").ap()
    tvas_d = nc.dram_tensor("tvas", [RPC * T, 4], F32, kind="ExternalInput").ap()
    vec_d = nc.dram_tensor("vec", [P, 48], F32, kind="ExternalInput").ap()
    out_d = nc.dram_tensor("out", [P, CH], F32, kind="ExternalOutput").ap()

    # vec columns:
    #  0: 4   -x per chunk          4: 8   -y per chunk      8:12  v per chunk
    # 12:16   t_max per chunk      16:20   integral_station  20:24 integral_speed
    # 24 kp5=5*station_kp  25 station_ki  26 lokp3=3*low_kp  27 low_ki
    # 28 dkp3=3*(high_kp-low_kp)  29 dki=high_ki-low_ki  30 -2*switch_speed
    # 32:48 identity-16 mask for the indirect_copy diagonal extract

    with tile.TileContext(nc) as tc:
        from contextlib import ExitStack

        with ExitStack() as ctx:
            singles = ctx.enter_context(tc.tile_pool(name="singles", bufs=1))
            stream = ctx.enter_context(tc.tile_pool(name="stream", bufs=2))
            work = ctx.enter_context(tc.tile_pool(name="work", bufs=2))
            small = ctx.enter_context(tc.tile_pool(name="small", bufs=2))
            accp = ctx.enter_context(tc.tile_pool(name="accp", bufs=1))

            vec = singles.tile([P, 48], F32)
            nc.sync.dma_start(out=vec[:], in_=vec_d)

            # rbcu[:, c] = p*T + c*128*T  (tvas row base for gather1, int32)
            # rbm1[:, c] = p*T + c*128*T - 1  (base with the ii=cnt-1 folded in)
            rbcu = singles.tile([P, CH], U32)
            rbm1 = singles.tile([P, CH], I32)
            for c in range(CH):
                nc.gpsimd.iota(
                    rbcu[:, c : c + 1], pattern=[[1, 1]],
                    base=c * P * T, channel_multiplier=T,
                )
                nc.gpsimd.iota(
                    rbm1[:, c : c + 1], pattern=[[1, 1]],
                    base=c * P * T - 1, channel_multiplier=T,
                )

            # per-core accumulators
            idx_all = accp.tile([P, 8 * CH], U32)  # FIND_INDEX8 outputs
            off1_all = accp.tile([P, CH], U32)
            G1 = accp.tile([P, 4 * CH], F32)  # (t,v,a,s) at idx, per chunk
            S_all = accp.tile([P, CH], F32)
            tcl_all = accp.tile([P, CH], F32)
            G2 = accp.tile([P, 8 * CH], F32)  # (t,v,a,s) at ii, ii+1
            G1r = G1[:].rearrange("p (c k) -> p c k", k=4)
            G2r = G2[:].rearrange("p (c k) -> p c k", k=8)

            rt_tiles = {}

            def heavy(c):
                """stream + dist2 + argmin for chunk c (dense big-op section)"""
                rows = slice(c * P, (c + 1) * P)
                xym_t = stream.tile([P, 2, T], F32, tag="xym", bufs=3, name="xym_t")
                nc.sync.dma_start(out=xym_t[:], in_=xym_d[rows])
                rt_t = stream.tile([P, T], F32, tag="rt", bufs=CH, name="rt_t")
                nc.sync.dma_start(out=rt_t[:], in_=rt_d[rows])
                rt_tiles[c] = rt_t

                dx2 = work.tile([P, T], F32, tag="dx2", bufs=3, name="dx2")
                nc.scalar.activation(
                    dx2[:], xym_t[:, 0, :], AF.Square,
                    bias=vec[:, c : c + 1], scale=1.0,
                )
                dy2 = work.tile([P, T], F32, tag="dy2", bufs=3, name="dy2")
                nc.scalar.activation(
                    dy2[:], xym_t[:, 1, :], AF.Square,
                    bias=vec[:, 4 + c : 5 + c], scale=1.0,
                )
                dist2 = work.tile([P, T], F32, tag="dist2", bufs=3, name="dist2")
                nc.vector.tensor_tensor(
                    out=dist2[:], in0=dx2[:], in1=dy2[:], op=OP.add
                )
                minv = small.tile([P, 1], F32, tag="minv", name="minv")
                nc.vector.tensor_reduce(
                    out=minv[:], in_=dist2[:], axis=mybir.AxisListType.X, op=OP.min
                )
                find_inst = nc.vector.max_index(
                    idx_all[:, 8 * c : 8 * c + 8],
                    minv[:, 0:1].to_broadcast([P, 8]),
                    dist2[:],
                )
                # gather1 offset, then issue gather1 immediately: its ~5us
                # indirect-DMA latency hides under the next two chunks' heavy
                # ops (tail(c) is emitted two heavies later)
                nc.vector.tensor_tensor(
                    out=off1_all[:, c : c + 1], in0=idx_all[:, 8 * c : 8 * c + 1],
                    in1=rbcu[:, c : c + 1], op=OP.add,
                )
                nc.gpsimd.indirect_dma_start(
                    out=G1[:, 4 * c : 4 * c + 4],
                    out_offset=None,
                    in_=tvas_d,
                    in_offset=IndirectOffsetOnAxis(ap=off1_all[:, c : c + 1], axis=0),
                )
                return find_inst

            def tail(c, after=None):
                """t_cl -> sign count -> gather2 for chunk c; G1[c] is already
                resident (issued in heavy(c)), so nothing here waits on a
                same-chunk gather round trip"""
                # t_cl = min(t_m + 0.8, t_max)  [t_m >= 0 so the max(.,0) is dead]
                tcl_inst = nc.vector.tensor_scalar(
                    out=tcl_all[:, c : c + 1],
                    in0=G1[:, 4 * c : 4 * c + 1],
                    scalar1=PREVIEW_WINDOW,
                    scalar2=vec[:, 12 + c : 13 + c],
                    op0=OP.add,
                    op1=OP.min,
                )
                if after is not None:
                    # ordering-only edge: the cost model underestimates the
                    # indirect-DMA latency, so without this the scheduler slots
                    # tcl before later chunks' heavy DVE ops and the in-order
                    # DVE stream stalls on the gather semaphore
                    tile.add_dep_helper(
                        tcl_inst.ins, after.ins, sync=False,
                        reason="keep tail after later heavy",
                    )
                # S = sum_j sign(t_cl - t_j)
                cntscr = work.tile([P, T], F32, tag="dx2", bufs=3, name="cntscr")
                nc.scalar.activation(
                    cntscr[:], rt_tiles[c][:], AF.Sign,
                    bias=tcl_all[:, c : c + 1], scale=-1.0,
                    accum_out=S_all[:, c : c + 1],
                )
                # cnt = f*(S/2-1024) + (S/2+1024), f = (t_cl >= t_max)
                dlt = small.tile([P, 1], F32, tag="dlt", name="dlt")
                nc.vector.tensor_scalar(
                    out=dlt[:], in0=S_all[:, c : c + 1], scalar1=0.5,
                    scalar2=-float(T // 2), op0=OP.mult, op1=OP.add,
                )
                a_t = small.tile([P, 1], F32, tag="a_t", name="a_t")
                nc.vector.tensor_scalar(
                    out=a_t[:], in0=S_all[:, c : c + 1], scalar1=0.5,
                    scalar2=float(T // 2), op0=OP.mult, op1=OP.add,
                )
                e_t = small.tile([P, 1], F32, tag="e_t", name="e_t")
                nc.vector.tensor_scalar(
                    out=e_t[:], in0=tcl_all[:, c : c + 1],
                    scalar1=vec[:, 12 + c : 13 + c], scalar2=dlt[:, 0:1],
                    op0=OP.is_ge, op1=OP.mult,
                )
                cnt = small.tile([P, 1], F32, tag="cnt", name="cnt")
                nc.vector.tensor_tensor(
                    out=cnt[:], in0=e_t[:], in1=a_t[:], op=OP.add
                )
                # off2 = (cnt - 1) + rowbase, via rbm1; cnt >= 1 always
                cnti = small.tile([P, 1], I32, tag="cnti", name="cnti")
                nc.vector.tensor_copy(cnti[:], cnt[:])
                off2 = small.tile([P, 1], I32, tag="off2", name="off2")
                nc.vector.tensor_tensor(
                    out=off2[:], in0=cnti[:], in1=rbm1[:, c : c + 1], op=OP.add
                )
                nc.gpsimd.indirect_dma_start(
                    out=G2[:, 8 * c : 8 * c + 8],
                    out_offset=None,
                    in_=tvas_d,
                    in_offset=IndirectOffsetOnAxis(ap=off2[:, 0:1], axis=0),
                )

            # software-pipelined emission (depth 2): chunk c's tail is issued
            # two heavies later so per-engine in-order streams never stall on
            # gather round-trips
            finds = [None] * CH
            finds[0] = heavy(0)
            finds[1] = heavy(1)
            finds[2] = heavy(2)
            tail(0, after=finds[2])
            finds[3] = heavy(3)
            tail(1, after=finds[3])
            tail(2)
            tail(3)

            # ---- phase F: batched frac + interpolation + PID ----
            # frac = clip((t_cl - t0) / (t1 - t0), 0, 1)   [t1-t0 ~ 0.1 > 0]
            t0v = G2r[:, :, 0]
            den = small.tile([P, CH], F32, tag="den")
            nc.vector.tensor_tensor(
                out=den[:], in0=G2r[:, :, 4], in1=t0v, op=OP.subtract
            )
            rec = small.tile([P, CH], F32, tag="rec")
            nc.vector.reciprocal(rec[:], den[:])
            num = small.tile([P, CH], F32, tag="num")
            nc.vector.tensor_tensor(
                out=num[:], in0=tcl_all[:], in1=t0v, op=OP.subtract
            )
            fr = small.tile([P, CH], F32, tag="fr")
            nc.vector.tensor_tensor(out=fr[:], in0=num[:], in1=rec[:], op=OP.mult)
            frac_all = small.tile([P, CH], F32, tag="frac_all")
            nc.vector.tensor_scalar(
                out=frac_all[:], in0=fr[:], scalar1=0.0, scalar2=1.0,
                op0=OP.max, op1=OP.min,
            )

            Dall = accp.tile([P, 4 * CH], F32)
            Dr = Dall[:].rearrange("p (c k) -> p c k", k=4)
            nc.vector.tensor_tensor(
                out=Dr, in0=G2r[:, :, 4:8], in1=G2r[:, :, 0:4], op=OP.subtract
            )
            Pall = accp.tile([P, 4 * CH], F32)
            for c in range(CH):
                nc.vector.tensor_scalar(
                    out=Pall[:, 4 * c : 4 * c + 4],
                    in0=Dall[:, 4 * c : 4 * c + 4],
                    scalar1=frac_all[:, c : c + 1],
                    scalar2=None,
                    op0=OP.mult,
                )
            Iall = accp.tile([P, 4 * CH], F32)
            Ir = Iall[:].rearrange("p (c k) -> p c k", k=4)
            Pr = Pall[:].rearrange("p (c k) -> p c k", k=4)
            nc.vector.tensor_tensor(
                out=Ir, in0=Pr, in1=G2r[:, :, 0:4], op=OP.add
            )

            s_m = G1r[:, :, 3]  # [P, CH] strided view
            v_p = Ir[:, :, 1]
            a_p = Ir[:, :, 2]
            s_p = Ir[:, :, 3]

            def pt(tag):
                return small.tile([P, CH], F32, tag=tag, name=tag)

            # station PI: station_err = 5*tanh((s_p-s_m)/5), folded as th*5
            serr0 = pt("serr0")
            nc.vector.tensor_tensor(out=serr0[:], in0=s_p, in1=s_m, op=OP.subtract)
            th = pt("th")
            nc.scalar.activation(
                th[:], serr0[:], AF.Tanh, scale=float(1.0 / STATION_ERR_LIM)
            )
            t1a = pt("t1a")  # station_err*DT = th*0.1
            nc.scalar.activation(t1a[:], th[:], AF.Identity, scale=0.1)
            ints0 = pt("ints0")
            nc.vector.tensor_tensor(
                out=ints0[:], in0=t1a[:], in1=vec[:, 16:20], op=OP.add
            )
            ints = pt("ints")
            nc.vector.tensor_scalar(
                out=ints[:], in0=ints0[:], scalar1=-INTEGRATOR_SAT,
                scalar2=INTEGRATOR_SAT, op0=OP.max, op1=OP.min,
            )
            so1 = pt("so1")  # station_kp*station_err = th*(5*station_kp)
            nc.scalar.activation(so1[:], th[:], AF.Identity, scale=vec[:, 24:25])
            so2 = pt("so2")
            nc.scalar.activation(so2[:], ints[:], AF.Identity, scale=vec[:, 25:26])
            soff = pt("soff")
            nc.vector.tensor_tensor(out=soff[:], in0=so1[:], in1=so2[:], op=OP.add)

            # speed PI: speed_err = 3*tanh(ve1/3) folded as th2*3
            ve0 = pt("ve0")
            nc.vector.tensor_tensor(out=ve0[:], in0=v_p, in1=soff[:], op=OP.add)
            ve1 = pt("ve1")
            nc.vector.tensor_tensor(
                out=ve1[:], in0=ve0[:], in1=vec[:, 8:12], op=OP.subtract
            )
            th2 = pt("th2")
            nc.scalar.activation(
                th2[:], ve1[:], AF.Tanh, scale=float(1.0 / SPEED_INPUT_LIM)
            )
            t2a = pt("t2a")  # speed_err*DT = th2*0.06
            nc.scalar.activation(t2a[:], th2[:], AF.Identity, scale=0.06)
            insp0 = pt("insp0")
            nc.vector.tensor_tensor(
                out=insp0[:], in0=t2a[:], in1=vec[:, 20:24], op=OP.add
            )
            insp = pt("insp")
            nc.vector.tensor_scalar(
                out=insp[:], in0=insp0[:], scalar1=-INTEGRATOR_SAT,
                scalar2=INTEGRATOR_SAT, op0=OP.max, op1=OP.min,
            )
            w = pt("w")
            nc.scalar.activation(
                w[:], vec[:, 8:12], AF.Sigmoid, bias=vec[:, 30:31], scale=2.0
            )
            kp3 = pt("kp3")  # 3*kp = w*dkp3 + lokp3
            nc.scalar.activation(
                kp3[:], w[:], AF.Identity, scale=vec[:, 28:29], bias=vec[:, 26:27]
            )
            ki = pt("ki")
            nc.scalar.activation(
                ki[:], w[:], AF.Identity, scale=vec[:, 29:30], bias=vec[:, 27:28]
            )
            p1 = pt("p1")  # kp*speed_err = kp3*th2
            nc.vector.tensor_tensor(out=p1[:], in0=kp3[:], in1=th2[:], op=OP.mult)
            p2 = pt("p2")
            nc.vector.tensor_tensor(out=p2[:], in0=ki[:], in1=insp[:], op=OP.mult)
            p3 = pt("p3")
            nc.vector.tensor_tensor(out=p3[:], in0=p1[:], in1=p2[:], op=OP.add)
            p4 = pt("p4")
            nc.vector.tensor_tensor(out=p4[:], in0=p3[:], in1=a_p, op=OP.add)
            accf = pt("accf")
            nc.vector.tensor_scalar(
                out=accf[:], in0=p4[:], scalar1=ACC_MIN, scalar2=ACC_MAX,
                op0=OP.max, op1=OP.min,
            )
            nc.sync.dma_start(out=out_d, in_=accf[:])

    nc.compile()
    _CACHE["nc"] = nc
    return nc


def _prepare_in_maps(inputs):
    def f(name):
        return np.ascontiguousarray(np.asarray(inputs[name], dtype=np.float32))

    rx = f("ref_x")
    ry = f("ref_y")
    rt = f("ref_t")
    valid = f("valid_mask")
    ym = np.where(valid > 0.5, ry, np.float32(MASK_BIG)).astype(np.float32)
    xym = np.stack([rx, ym], axis=1)  # [B, 2, T] contiguous
    tvas = np.stack(
        [rt, f("ref_v"), f("ref_a"), f("ref_s")], axis=2
    )  # [B, T, 4] contiguous

    xs = f("x")
    ys = f("y")
    vs = f("v")
    tmax = f("t_max")
    ist = f("integral_station")
    isp = f("integral_speed")

    sk = np.float32(np.asarray(inputs["station_kp"]))
    si = np.float32(np.asarray(inputs["station_ki"]))
    lkp = np.float32(np.asarray(inputs["low_speed_kp"]))
    lki = np.float32(np.asarray(inputs["low_speed_ki"]))
    hkp = np.float32(np.asarray(inputs["high_speed_kp"]))
    hki = np.float32(np.asarray(inputs["high_speed_ki"]))
    sw = np.float32(np.asarray(inputs["switch_speed"]))

    in_maps = []
    for core in range(NCORES):
        base = core * RPC
        sl = slice(base, base + RPC)
        vec = np.zeros((P, 48), np.float32)
        for c in range(CH):
            rows = slice(base + c * P, base + (c + 1) * P)
            vec[:, 0 + c] = -xs[rows]
            vec[:, 4 + c] = -ys[rows]
            vec[:, 8 + c] = vs[rows]
            vec[:, 12 + c] = tmax[rows]
            vec[:, 16 + c] = ist[rows]
            vec[:, 20 + c] = isp[rows]
        vec[:, 24] = np.float32(5.0) * sk
        vec[:, 25] = si
        vec[:, 26] = np.float32(3.0) * lkp
        vec[:, 27] = lki
        vec[:, 28] = np.float32(3.0) * (hkp - lkp)
        vec[:, 29] = hki - lki
        vec[:, 30] = np.float32(-2.0) * sw
        vec[np.arange(P), 32 + (np.arange(P) % 16)] = 1.0
        in_maps.append(
            {
                "xym": xym[sl],
                "rt": np.ascontiguousarray(rt[sl]),
                "tvas": tvas[sl].reshape(RPC * T, 4),
                "vec": vec,
            }
        )
    return in_maps


def _assemble(results):
    out = np.empty(B, np.float32)
    for core in range(NCORES):
        oc = np.asarray(results[core]["out"], np.float32)  # [P, CH]
        out[core * RPC : (core + 1) * RPC] = oc.T.reshape(RPC)
    return out


def kernel(**inputs):
    nc = _build_program()
    in_maps = _prepare_in_maps(inputs)
    res = run_bass_kernel_spmd(nc, in_maps, core_ids=list(range(NCORES)))
    return _assemble(res.results)


def kernel_traced(inputs, **kwargs):
    """For test.py: same as kernel() but returns (output, BassKernelResults)."""
    nc = _build_program()
    in_maps = _prepare_in_maps(inputs)
    res = run_bass_kernel_spmd(
        nc, in_maps, core_ids=list(range(NCORES)), trace=True, **kwargs
    )
    return _assemble(res.results), res
